# revision 13
# baseline (speedup 1.0000x reference)
"""AutoCov1D Trainium2 kernel (8 NeuronCores, data-parallel over batch).

Math: for window n (stride 8, width 64), with X1 = X[:, :-64], X2 = X[:, 64:]:
  p1 = einsum('bnw,wdc', X1win, Wgt); p2 likewise with X2win
  out = mean_d(p1c * p2c) + bias   (p*c centered over d)

Two exact simplifications used here:
  1. Centering over d is linear in the weight, so pre-center the weight:
     Wtil = (W - mean_d W) / sqrt(D); then no mean terms remain.
  2. X2 windows are X1 windows shifted by 8 window indices (64 = 8*stride),
     so ONE projection P[b,m,:] = sum_w X[b, 8m+w] * Wtil[w,:] over m=0..504
     serves both operands:  out[b,n,c] = sum_d P[b,n,d,c]*P[b,n+8,d,c] + bias.

Per-core layout (B_shard=4): weight-stationary matmuls with K=w(64),
PSUM tile P_d[c=128 partitions, m=505] per (b, d); covariance products and
the d-accumulation on DVE; evacuation copies on ACT; bias on GPSIMD.
"""

import sys

import numpy as np

if "/opt/trn_rl_repo" not in sys.path:
    sys.path.insert(0, "/opt/trn_rl_repo")

_B, _T, _W, _D, _C = 32, 4096, 64, 32, 128
_NCORES = 8
_BSH = _B // _NCORES  # 4
_M = 505  # projection windows per batch row
_MM = 506  # matmul free dim (even, fp32r ISA requirement; last col unused)
_N = 497  # output windows per batch row
_S = 8  # stride
_XLEN = _M * _S + 8  # 4048, padded per-partition X span

_NC_CACHE = None


def _build_nc():
    import concourse.bass as bass
    import concourse.tile as tile
    from concourse import bacc, mybir
    from contextlib import ExitStack

    f32 = mybir.dt.float32
    f32r = mybir.dt.float32r
    bf16 = mybir.dt.bfloat16

    nc = bacc.Bacc(None, target_bir_lowering=False)
    x = nc.declare_dram_parameter("xsh", [_BSH, _W, _XLEN], f32r, isOutput=False)
    wt = nc.declare_dram_parameter("wt", [_W, _D, _C], f32r, isOutput=False)
    bias = nc.declare_dram_parameter("bias", [_C, 1], f32, isOutput=False)
    out = nc.declare_dram_parameter("out", [_BSH, _C, _N], f32, isOutput=True)

    with ExitStack() as ctx:
        tc = ctx.enter_context(tile.TileContext(nc))
        singles = ctx.enter_context(tc.tile_pool(name="singles", bufs=1))
        xpool = ctx.enter_context(tc.tile_pool(name="xpool", bufs=2))
        psp = ctx.enter_context(tc.tile_pool(name="psp", bufs=2, space="PSUM"))
        evacp = ctx.enter_context(tc.tile_pool(name="evacp", bufs=6))
        prodp = ctx.enter_context(tc.tile_pool(name="prodp", bufs=2))
        treep = ctx.enter_context(tc.tile_pool(name="treep", bufs=2))
        accp = ctx.enter_context(tc.tile_pool(name="accp", bufs=2))
        outp = ctx.enter_context(tc.tile_pool(name="outp", bufs=2))

        wt_sb = singles.tile([_W, _D, _C], f32r)
        nc.sync.dma_start(out=wt_sb, in_=wt[:, :, :])
        bias_sb = singles.tile([_C, 1], f32)
        nc.sync.dma_start(out=bias_sb, in_=bias[:, :])

        for b in range(_BSH):
            xsh = xpool.tile([_W, _XLEN], f32r)
            nc.sync.dma_start(out=xsh, in_=x[b])
            # strided view: rhs[w, m] = X[b, 8m + w]
            xview = xsh.rearrange("p (m s) -> p m s", s=_S)
            prodb = prodp.tile([_C, _D, _N], bf16)
            for dq in range(_D // 4):
                ps = psp.tile([_C, 4, 512], f32)
                for j in range(4):
                    nc.tensor.matmul(
                        ps[:, j, 0:_MM],
                        lhsT=wt_sb[:, 4 * dq + j, :],
                        rhs=xview[:, 0:_MM, 0],
                        start=True,
                        stop=True,
                    )
                ev = evacp.tile([_C, 4, _MM], bf16)
                nc.scalar.copy(out=ev[:, :, :], in_=ps[:, :, 0:_MM])
                nc.vector.tensor_mul(
                    prodb[:, 4 * dq : 4 * dq + 4, :],
                    ev[:, :, 0:_N],
                    ev[:, :, _S:_M],
                )
            # tree reduction over d (bf16, 2x mode, batched)
            tA = treep.tile([_C, 16, _N], bf16, tag="tA")
            tB = treep.tile([_C, 8, _N], bf16, tag="tB")
            nc.vector.tensor_add(tA[:, :, :], prodb[:, 0:16, :], prodb[:, 16:32, :])
            nc.vector.tensor_add(tB[:, :, :], tA[:, 0:8, :], tA[:, 8:16, :])
            nc.vector.tensor_add(tA[:, 0:4, :], tB[:, 0:4, :], tB[:, 4:8, :])
            nc.vector.tensor_add(tB[:, 0:2, :], tA[:, 0:2, :], tA[:, 2:4, :])
            acc = accp.tile([_C, _N], bf16)
            nc.vector.tensor_add(acc[:, :], tB[:, 0, :], tB[:, 1, :])
            ot = outp.tile([_C, _N], f32)
            nc.vector.tensor_scalar_add(ot[:, :], acc[:, :], bias_sb[:, 0:1])
            nc.sync.dma_start(out=out[b], in_=ot[:, :])
    nc.finalize()
    return nc


def _prep_inputs(X, weight, bias):
    X = np.asarray(X, dtype=np.float32)
    weight = np.asarray(weight, dtype=np.float32)
    bias = np.asarray(bias, dtype=np.float32)

    wtil = (weight - weight.mean(axis=1, keepdims=True)) / np.sqrt(np.float32(_D))
    wtil = np.ascontiguousarray(wtil, dtype=np.float32)

    # xsh[b, w, i] = X[b, w + i]  (zero-padded past T)
    xsh = np.zeros((_B, _W, _XLEN), dtype=np.float32)
    for w in range(_W):
        n = min(_XLEN, _T - w)
        xsh[:, w, :n] = X[:, w : w + n]

    bias2 = np.ascontiguousarray(bias.reshape(_C, 1))

    in_maps = []
    for k in range(_NCORES):
        in_maps.append(
            {
                "xsh": np.ascontiguousarray(xsh[k * _BSH : (k + 1) * _BSH]),
                "wt": wtil,
                "bias": bias2,
            }
        )
    return in_maps


def get_nc():
    global _NC_CACHE
    if _NC_CACHE is None:
        _NC_CACHE = _build_nc()
    return _NC_CACHE


def run(X, weight, bias, trace=False, tmpdir=None):
    """Returns (full_output, BassKernelResults)."""
    from concourse.bass_utils import run_bass_kernel_spmd

    nc = get_nc()
    in_maps = _prep_inputs(X, weight, bias)
    res = run_bass_kernel_spmd(
        nc, in_maps, core_ids=list(range(_NCORES)), trace=trace, tmpdir=tmpdir
    )
    parts = [res.results[i]["out"].transpose(0, 2, 1) for i in range(_NCORES)]
    full = np.ascontiguousarray(np.concatenate(parts, axis=0), dtype=np.float32)
    return full, res


def kernel(X, weight, bias):
    full, _ = run(X, weight, bias)
    return full
